# revision 2
# baseline (speedup 1.0000x reference)
"""Trainium2 Bass kernel for the AP-loss metric (nn_APLoss).

Computes, for N=262144 logits with the first FG=2048 being positives:
  threshold-masked average-precision surrogate metric
    metric = 1 - mean_i(cummax-in-sorted-order(a_i / (a_i + b_i)))
  with a_i = 0.5 + sum_fg clip((fg - v_i)/2 + .5, 0, 1),
       b_i = sum_{bg valid} clip((bg - v_i)/2 + .5, 0, 1).

Key identities used on device:
  sum_j relu(u_j + c) = sum_j max(u_j, -c) + n*c          (VectorE 1-instr form)
  clip(x,0,1) = relu(x) - relu(x-1)
  bg < min(fg)-1 contributes exactly 0 to every query (threshold mask is
  redundant for the sums); invalid bg (target!=0) pinned to u=-500 -> 0.
The cummax over the sorted-fg scan is order-free: prec_i = max{cur_j : v_j <= v_i}.

Sharding: bg (260096) and fg-data (2048) split across 8 cores; all 2048
queries on every core; one 16KB AllReduce of partial sums; postprocessing
replicated on every core.
"""

import os
import sys

import numpy as np

sys.path.insert(0, "/opt/trn_rl_repo")

P = 128
FG = 2048
N = 262144
BG = N - FG
NCORES = 8
QB = FG // P           # 16 query blocks
FGS = FG // NCORES     # 256 fg data elems per core
BGS = BG // NCORES     # 32512 bg elems per core
DATA = FGS + BGS       # 32768 combined data elems per core
NCH = 4
CL = DATA // NCH       # 8192 chunk length
# ScalarE unit cost ~ (CL+352)/1.2GHz ; VectorE fp16 4x ~ (CL/4+60)/0.96GHz
N_UNITS = NCH * QB * 2
N_SCALAR_UNITS = 29    # balance split of the 128 main-loop units

_compiled = None


def _scalar_unit(u):
    # Spread N_SCALAR_UNITS evenly over the N_UNITS unit indices.
    return (u * N_SCALAR_UNITS) // N_UNITS != ((u + 1) * N_SCALAR_UNITS) // N_UNITS


def _build():
    import concourse.bacc as bacc
    import concourse.tile as tile
    from concourse import mybir

    F32 = mybir.dt.float32
    F16 = mybir.dt.float16
    I32 = mybir.dt.int32
    ALU = mybir.AluOpType
    AF = mybir.ActivationFunctionType

    nc = bacc.Bacc("TRN2", target_bir_lowering=False, debug=False,
                   num_devices=NCORES)
    fgq_e = nc.declare_dram_parameter("fgq", [FG], F32, isOutput=False)
    fgsh_e = nc.declare_dram_parameter("fgsh", [FGS], F32, isOutput=False)
    bgsh_e = nc.declare_dram_parameter("bgsh", [BGS], F32, isOutput=False)
    tgt_e = nc.declare_dram_parameter("tgt", [BGS], I32, isOutput=False)
    out_e = nc.declare_dram_parameter("out", [1, 1], F32, isOutput=True)

    # unit -> engine assignment, and per-(blk, shift) VectorE chunk counts
    nv = np.zeros((QB, 2), dtype=np.int64)
    unit_engine = {}
    u = 0
    for ch in range(NCH):
        for blk in range(QB):
            for s in range(2):
                on_s = _scalar_unit(u)
                unit_engine[(ch, blk, s)] = "S" if on_s else "V"
                if not on_s:
                    nv[blk, s] += 1
                u += 1

    with tile.TileContext(nc) as tc:
        with tc.tile_pool(name="sbuf", bufs=1) as sb, \
             tc.tile_pool(name="dram", bufs=1, space="DRAM") as dram, \
             tc.tile_pool(name="psum", bufs=1, space="PSUM") as ps:

            # ---------------- inputs ----------------
            F = sb.tile([P, QB], F32)          # all 2048 fg; query i at (p, blk) = p*QB+blk
            nc.sync.dma_start(out=F[:], in_=fgq_e[:].rearrange("(p c) -> p c", p=P))
            FS = sb.tile([P, FGS // P], F32)   # this core's fg data shard
            nc.sync.dma_start(out=FS[:], in_=fgsh_e[:].rearrange("(p c) -> p c", p=P))
            B = sb.tile([P, BGS // P], F32)
            nc.sync.dma_start(out=B[:], in_=bgsh_e[:].rearrange("(p c) -> p c", p=P))
            T = sb.tile([P, BGS // P], I32)
            nc.sync.dma_start(out=T[:], in_=tgt_e[:].rearrange("(p c) -> p c", p=P))

            # ---------------- query tiles ----------------
            negc0 = sb.tile([P, QB], F32)      # -c0 = 0.5 v - 0.5
            nc.vector.tensor_scalar(out=negc0[:], in0=F[:], scalar1=0.5,
                                    scalar2=-0.5, op0=ALU.mult, op1=ALU.add)
            negc1 = sb.tile([P, QB], F32)      # -c1 = -c0 + 1
            nc.vector.tensor_scalar(out=negc1[:], in0=negc0[:], scalar1=1.0,
                                    scalar2=None, op0=ALU.add)
            c0 = sb.tile([P, QB], F32)
            nc.vector.tensor_scalar(out=c0[:], in0=negc0[:], scalar1=-1.0,
                                    scalar2=None, op0=ALU.mult)
            c1 = sb.tile([P, QB], F32)
            nc.vector.tensor_scalar(out=c1[:], in0=c0[:], scalar1=-1.0,
                                    scalar2=None, op0=ALU.add)

            # ---------------- preprocess data -> u fp16 ----------------
            # u_bg = target==0 ? 0.5*bg : -500   (=-500 -> contributes 0)
            M = sb.tile([P, BGS // P], F32)
            nc.vector.tensor_scalar(out=M[:], in0=T[:], scalar1=0,
                                    scalar2=None, op0=ALU.is_equal)
            T1 = sb.tile([P, BGS // P], F32)
            nc.vector.tensor_scalar(out=T1[:], in0=B[:], scalar1=0.5,
                                    scalar2=500.0, op0=ALU.mult, op1=ALU.add)
            T2 = sb.tile([P, BGS // P], F32)
            nc.vector.tensor_tensor(out=T2[:], in0=T1[:], in1=M[:], op=ALU.mult)
            UB16 = sb.tile([P, BGS // P], F16)
            nc.vector.tensor_scalar(out=UB16[:], in0=T2[:], scalar1=-500.0,
                                    scalar2=None, op0=ALU.add)
            UF16 = sb.tile([P, FGS // P], F16)
            nc.vector.tensor_scalar(out=UF16[:], in0=FS[:], scalar1=0.5,
                                    scalar2=None, op0=ALU.mult)

            udata = dram.tile([DATA], F16)
            nc.sync.dma_start(out=udata[0:FGS].rearrange("(p c) -> p c", p=P),
                              in_=UF16[:])
            nc.sync.dma_start(out=udata[FGS:DATA].rearrange("(p c) -> p c", p=P),
                              in_=UB16[:])

            # ---------------- broadcast data across partitions ----------------
            UFGB = sb.tile([P, FGS], F16)
            nc.sync.dma_start(
                out=UFGB[:],
                in_=udata[0:FGS].unsqueeze(0).broadcast_to([P, FGS]))
            UBC = []
            for ch in range(NCH):
                t = sb.tile([P, CL], F16, name=f"ubc{ch}")
                nc.sync.dma_start(
                    out=t[:],
                    in_=udata[ch * CL:(ch + 1) * CL].unsqueeze(0)
                    .broadcast_to([P, CL]))
                UBC.append(t)

            # ---------------- main loop ----------------
            ACC = [sb.tile([P, QB * NCH], F32, name=f"acc{s}") for s in range(2)]
            AFA = [sb.tile([P, QB], F32, name=f"afa{s}") for s in range(2)]
            SCRV = sb.tile([P, CL], F16)
            SCRS = sb.tile([P, CL], F16)
            SCRF = sb.tile([P, FGS], F16)

            negc = [negc0, negc1]
            cc = [c0, c1]
            for ch in range(NCH):
                for blk in range(QB):
                    for s in range(2):
                        acc_ap = ACC[s][:, blk * NCH + ch:blk * NCH + ch + 1]
                        if unit_engine[(ch, blk, s)] == "V":
                            nc.vector.tensor_scalar(
                                out=SCRV[:], in0=UBC[ch][:],
                                scalar1=negc[s][:, blk:blk + 1], scalar2=None,
                                op0=ALU.max, op1=ALU.add, accum_out=acc_ap)
                        else:
                            nc.scalar.activation(
                                out=SCRS[:], in_=UBC[ch][:], func=AF.Relu,
                                bias=cc[s][:, blk:blk + 1], scale=1.0,
                                accum_out=acc_ap)
            # fg-data part: all on VectorE (max form)
            for blk in range(QB):
                for s in range(2):
                    nc.vector.tensor_scalar(
                        out=SCRF[:], in0=UFGB[:],
                        scalar1=negc[s][:, blk:blk + 1], scalar2=None,
                        op0=ALU.max, op1=ALU.add,
                        accum_out=AFA[s][:, blk:blk + 1])

            # ---------------- combine partials ----------------
            asum = []
            for s in range(2):
                t = sb.tile([P, QB], F32, name=f"asum{s}")
                nc.vector.tensor_reduce(
                    out=t[:], in_=ACC[s][:].rearrange("p (b c) -> p b c", c=NCH),
                    axis=mybir.AxisListType.X, op=ALU.add)
                asum.append(t)
            D = sb.tile([P, QB], F32)
            nc.vector.tensor_sub(D[:], asum[0][:], asum[1][:])
            # correction: comb += CL * (nv0*c0 - nv1*c1)
            NV0 = sb.tile([P, QB], F32)
            NV1 = sb.tile([P, QB], F32)
            for blk in range(QB):
                nc.vector.memset(NV0[:, blk:blk + 1], float(nv[blk, 0]))
                nc.vector.memset(NV1[:, blk:blk + 1], float(nv[blk, 1]))
            T0q = sb.tile([P, QB], F32)
            nc.vector.tensor_tensor(out=T0q[:], in0=c0[:], in1=NV0[:], op=ALU.mult)
            T1q = sb.tile([P, QB], F32)
            nc.vector.tensor_tensor(out=T1q[:], in0=c1[:], in1=NV1[:], op=ALU.mult)
            CT = sb.tile([P, QB], F32)
            nc.vector.tensor_sub(CT[:], T0q[:], T1q[:])
            CT2 = sb.tile([P, QB], F32)
            nc.vector.tensor_scalar(out=CT2[:], in0=CT[:], scalar1=float(CL),
                                    scalar2=None, op0=ALU.mult)
            COMB = sb.tile([P, QB], F32)
            nc.vector.tensor_add(COMB[:], D[:], CT2[:])
            # fg part fix: afix = (afa0 - afa1) + FGS   (both shifts VectorE)
            DF = sb.tile([P, QB], F32)
            nc.vector.tensor_sub(DF[:], AFA[0][:], AFA[1][:])
            AFIX = sb.tile([P, QB], F32)
            nc.vector.tensor_scalar(out=AFIX[:], in0=DF[:], scalar1=float(FGS),
                                    scalar2=None, op0=ALU.add)

            # ---------------- AllReduce ----------------
            cc_in = dram.tile([2 * FG], F32)
            cc_out = dram.tile([2 * FG], F32)
            nc.sync.dma_start(out=cc_in[0:FG].rearrange("(p c) -> p c", p=P),
                              in_=COMB[:])
            nc.sync.dma_start(out=cc_in[FG:2 * FG].rearrange("(p c) -> p c", p=P),
                              in_=AFIX[:])
            nc.gpsimd.collective_compute(
                "AllReduce", ALU.add,
                replica_groups=[list(range(NCORES))],
                ins=[cc_in.opt()], outs=[cc_out.opt()])
            CMB = sb.tile([P, QB], F32)
            nc.sync.dma_start(out=CMB[:],
                              in_=cc_out[0:FG].rearrange("(p c) -> p c", p=P))
            AFG = sb.tile([P, QB], F32)
            nc.sync.dma_start(out=AFG[:],
                              in_=cc_out[FG:2 * FG].rearrange("(p c) -> p c", p=P))

            # ---------------- cur = a / s ----------------
            Sq = sb.tile([P, QB], F32)
            nc.vector.tensor_scalar(out=Sq[:], in0=CMB[:], scalar1=0.5,
                                    scalar2=None, op0=ALU.add)
            Aq = sb.tile([P, QB], F32)
            nc.vector.tensor_scalar(out=Aq[:], in0=AFG[:], scalar1=0.5,
                                    scalar2=None, op0=ALU.add)
            RS = sb.tile([P, QB], F32)
            nc.vector.reciprocal(RS[:], Sq[:])
            CUR = sb.tile([P, QB], F32)
            nc.vector.tensor_tensor(out=CUR[:], in0=Aq[:], in1=RS[:], op=ALU.mult)

            # ---------------- prec = masked running max ----------------
            # fp16-rounded queries so self-comparison is always true
            VQH = sb.tile([P, QB], F16)
            nc.vector.tensor_copy(VQH[:], F[:])
            VQR = sb.tile([P, QB], F32)
            nc.vector.tensor_copy(VQR[:], VQH[:])
            CUR16 = sb.tile([P, QB], F16)
            nc.vector.tensor_copy(CUR16[:], CUR[:])

            vline = dram.tile([FG], F16)
            nc.sync.dma_start(out=vline[:].rearrange("(p c) -> p c", p=P),
                              in_=VQH[:])
            cline = dram.tile([FG], F16)
            nc.sync.dma_start(out=cline[:].rearrange("(p c) -> p c", p=P),
                              in_=CUR16[:])
            VB = sb.tile([P, FG], F16)
            nc.sync.dma_start(out=VB[:],
                              in_=vline[:].unsqueeze(0).broadcast_to([P, FG]))
            CB = sb.tile([P, FG], F16)
            nc.sync.dma_start(out=CB[:],
                              in_=cline[:].unsqueeze(0).broadcast_to([P, FG]))

            PREC = sb.tile([P, QB], F32)
            MSK = sb.tile([P, FG], F16)
            TM = sb.tile([P, FG], F16)
            for blk in range(QB):
                # (v_j > v_i) * -1e4 ; then + cur_j ; then reduce max
                nc.vector.tensor_scalar(out=MSK[:], in0=VB[:],
                                        scalar1=VQR[:, blk:blk + 1],
                                        scalar2=-10000.0,
                                        op0=ALU.is_gt, op1=ALU.mult)
                nc.vector.tensor_tensor(out=TM[:], in0=MSK[:], in1=CB[:],
                                        op=ALU.add)
                nc.vector.tensor_reduce(out=PREC[:, blk:blk + 1], in_=TM[:],
                                        axis=mybir.AxisListType.X, op=ALU.max)

            # ---------------- metric = 1 - sum(prec)/FG ----------------
            PSUM = sb.tile([P, 1], F32)
            nc.vector.tensor_reduce(out=PSUM[:], in_=PREC[:],
                                    axis=mybir.AxisListType.X, op=ALU.add)
            ONES = sb.tile([P, 1], F32)
            nc.vector.memset(ONES[:], 1.0)
            TOT = ps.tile([1, 1], F32)
            nc.tensor.matmul(TOT[:], lhsT=PSUM[:, 0:1], rhs=ONES[:, 0:1],
                             start=True, stop=True)
            MT = sb.tile([1, 1], F32)
            nc.vector.tensor_scalar(out=MT[:], in0=TOT[0:1, 0:1],
                                    scalar1=-1.0 / FG, scalar2=1.0,
                                    op0=ALU.mult, op1=ALU.add)
            nc.sync.dma_start(out=out_e[:, :], in_=MT[:])
    nc.compile()
    return nc


def _get_compiled():
    global _compiled
    if _compiled is None:
        _compiled = _build()
    return _compiled


def kernel(logits, targets, _trace=False, _trace_kwargs=None):
    from concourse.bass_utils import run_bass_kernel_spmd

    logits = np.ascontiguousarray(np.asarray(logits), dtype=np.float32)
    targets = np.ascontiguousarray(np.asarray(targets), dtype=np.int32)
    fg = logits[:FG]
    bg = logits[FG:]
    tg = targets[FG:]
    in_maps = []
    for r in range(NCORES):
        in_maps.append({
            "fgq": fg,
            "fgsh": np.ascontiguousarray(fg[r * FGS:(r + 1) * FGS]),
            "bgsh": np.ascontiguousarray(bg[r * BGS:(r + 1) * BGS]),
            "tgt": np.ascontiguousarray(tg[r * BGS:(r + 1) * BGS]),
        })
    nc = _get_compiled()
    kw = {}
    if _trace:
        kw = dict(trace=True, **(_trace_kwargs or {}))
    res = run_bass_kernel_spmd(nc, in_maps, core_ids=list(range(NCORES)), **kw)
    out = np.float32(res.results[0]["out"][0, 0])
    if _trace:
        return out, res
    return out


if __name__ == "__main__":
    rng = np.random.default_rng(0)
    logits = rng.standard_normal(N).astype(np.float32)
    targets = np.concatenate([np.ones(FG, np.int32), np.zeros(BG, np.int32)])
    print("metric:", kernel(logits, targets))


# revision 8
# speedup vs baseline: 2.5846x; 2.5846x over previous
"""Trainium2 Bass kernel for the AP-loss metric (nn_APLoss).

For N=262144 logits with the first FG=2048 being positives:
    metric = 1 - mean_i(prec_i),  prec_i = max{cur_j : v_j <= v_i}
    cur_i = a_i / (a_i + b_i)
    a_i = 0.5 + sum_fg clip((fg - v_i)/2 + .5, 0, 1)
    b_i = sum_{bg: target==0} clip((bg - v_i)/2 + .5, 0, 1)
(The reference's sorted scan + cummax is order-free; its bg>=min(fg)-1
threshold mask is provably redundant for the sums.)

Per core (8-way shard of bg and of the fg-data):
  clip(u + c) with u = x/2, c_i = 0.5 - v_i/2 is evaluated 66.6M times via
  three engine paths:
  - PE path (query blocks 0..11): data kept natural [128 x 254]; per data
    column a 4x-mode fp16 tensor_scalar relu pass [128 x 1536] against a
    query-broadcast tile, min-with-1 passes batched over 8 columns, then
    TensorE sums along partitions with accumulating matmul chains into
    PSUM (direct clip sums).
  - ScalarE path (query blocks 12..15): activation(Relu, bias=c, accum_out)
    over partition-broadcast data, clip sum = relu-sum(c0) - relu-sum(c1).
  - The fg-data part (a_i, all 2048 queries x 256 elems) rides the PE path.
  One 16KB AllReduce of per-query partials; postprocessing (cur, masked
  running max, mean) replicated on every core.
"""

import os
import sys

import numpy as np

sys.path.insert(0, "/opt/trn_rl_repo")

P = 128
FG = 2048
N = 262144
BG = N - FG
NCORES = 8
QB = FG // P            # 16 query blocks
FGS = FG // NCORES      # 256 fg data elems per core
BGS = BG // NCORES      # 32512 bg elems per core
NCOL = BGS // P         # 254 data columns per core
QPE = 1536              # queries on the PE path (blocks 0..11)
NBPE = QPE // P         # 12
MMN = 512               # matmul moving-dim chunk (one PSUM bank)
BATCH = 6               # data columns per min-pass batch

_compiled = None


def _build():
    import concourse.bacc as bacc
    import concourse.tile as tile
    from concourse import mybir

    F32 = mybir.dt.float32
    F16 = mybir.dt.float16
    I32 = mybir.dt.int32
    ALU = mybir.AluOpType
    AF = mybir.ActivationFunctionType

    nc = bacc.Bacc("TRN2", target_bir_lowering=False, debug=False,
                   num_devices=NCORES)
    fgq_e = nc.declare_dram_parameter("fgq", [FG], F32, isOutput=False)
    fgsh_e = nc.declare_dram_parameter("fgsh", [FGS], F32, isOutput=False)
    bgsh_e = nc.declare_dram_parameter("bgsh", [BGS], F32, isOutput=False)
    tgt_e = nc.declare_dram_parameter("tgt", [BGS], I32, isOutput=False)
    out_e = nc.declare_dram_parameter("out", [1, 1], F32, isOutput=True)

    with tile.TileContext(nc) as tc:
        with tc.tile_pool(name="sbuf", bufs=1) as sb, \
             tc.tile_pool(name="dram", bufs=1, space="DRAM") as dram, \
             tc.tile_pool(name="psum", bufs=1, space="PSUM") as ps:

            # ---------------- inputs ----------------
            F = sb.tile([P, QB], F32)         # all 2048 fg; query i at (p, blk)=p*QB+blk
            nc.sync.dma_start(out=F[:], in_=fgq_e[:].rearrange("(p c) -> p c", p=P))
            FS = sb.tile([P, FGS // P], F32)  # this core's fg data shard
            nc.sync.dma_start(out=FS[:], in_=fgsh_e[:].rearrange("(p c) -> p c", p=P))
            B = sb.tile([P, NCOL], F32)
            nc.sync.dma_start(out=B[:], in_=bgsh_e[:].rearrange("(p c) -> p c", p=P))
            T = sb.tile([P, NCOL], I32)
            nc.sync.dma_start(out=T[:], in_=tgt_e[:].rearrange("(p c) -> p c", p=P))

            # ---------------- query tiles ----------------
            c0 = sb.tile([P, QB], F32)        # c0 = 0.5 - 0.5 v
            nc.vector.tensor_scalar(out=c0[:], in0=F[:], scalar1=-0.5,
                                    scalar2=0.5, op0=ALU.mult, op1=ALU.add)
            c1 = sb.tile([P, QB], F32)        # c1 = c0 - 1
            nc.vector.tensor_scalar(out=c1[:], in0=c0[:], scalar1=-1.0,
                                    scalar2=None, op0=ALU.add)
            c0h = sb.tile([P, QB], F16)
            nc.vector.tensor_copy(c0h[:], c0[:])

            # query-broadcast lines for the PE path
            cqpe_d = dram.tile([QPE], F16)    # c0 of blocks 0..11, (p,blk) order
            nc.sync.dma_start(out=cqpe_d[:].rearrange("(p c) -> p c", p=P),
                              in_=c0h[:, 0:NBPE])
            cqf_d = dram.tile([FG], F16)      # c0 of all blocks (for the fg part)
            nc.sync.dma_start(out=cqf_d[:].rearrange("(p c) -> p c", p=P),
                              in_=c0h[:])
            CQPE = sb.tile([P, QPE], F16)
            nc.sync.dma_start(out=CQPE[:],
                              in_=cqpe_d[:].unsqueeze(0).broadcast_to([P, QPE]))
            CQF = sb.tile([P, FG], F16)
            nc.sync.dma_start(out=CQF[:],
                              in_=cqf_d[:].unsqueeze(0).broadcast_to([P, FG]))

            # ---------------- data preprocess ----------------
            # u_bg = target==0 ? 0.5*bg : -500 (-500 -> clip contributes 0)
            M = sb.tile([P, NCOL], F32)
            nc.vector.tensor_scalar(out=M[:], in0=T[:], scalar1=0,
                                    scalar2=None, op0=ALU.is_equal)
            T1 = sb.tile([P, NCOL], F32)
            nc.vector.tensor_scalar(out=T1[:], in0=B[:], scalar1=0.5,
                                    scalar2=500.0, op0=ALU.mult, op1=ALU.add)
            T2 = sb.tile([P, NCOL], F32)
            nc.vector.tensor_tensor(out=T2[:], in0=T1[:], in1=M[:], op=ALU.mult)
            U32 = sb.tile([P, NCOL], F32)     # natural-layout data (PE-path scalars)
            nc.vector.tensor_scalar(out=U32[:], in0=T2[:], scalar1=-500.0,
                                    scalar2=None, op0=ALU.add)
            UF32 = sb.tile([P, FGS // P], F32)
            nc.vector.tensor_scalar(out=UF32[:], in0=FS[:], scalar1=0.5,
                                    scalar2=None, op0=ALU.mult)

            # fp16 copy -> HBM -> partition-broadcast (ScalarE path data)
            U16 = sb.tile([P, NCOL], F16)
            nc.vector.tensor_copy(U16[:], U32[:])
            udata = dram.tile([BGS], F16)
            nc.sync.dma_start(out=udata[:].rearrange("(p c) -> p c", p=P),
                              in_=U16[:])
            UBC = sb.tile([P, BGS], F16)
            nc.sync.dma_start(out=UBC[:],
                              in_=udata[:].unsqueeze(0).broadcast_to([P, BGS]))

            # ---------------- PE path: bg clip sums for blocks 0..11 ----------------
            ONES16 = sb.tile([P, 1], F16)
            nc.vector.memset(ONES16[:], 1.0)
            PSB = ps.tile([1, QPE], F32)      # 3 banks
            PSA = ps.tile([1, FG], F32)       # 4 banks
            Y = sb.tile([P, BATCH * QPE], F16)
            nbatch = (NCOL + BATCH - 1) // BATCH
            for bt in range(nbatch):
                cols = range(bt * BATCH, min((bt + 1) * BATCH, NCOL))
                ncols = len(cols)
                for k, col in enumerate(cols):
                    nc.vector.tensor_scalar(
                        out=Y[:, k * QPE:(k + 1) * QPE], in0=CQPE[:],
                        scalar1=U32[:, col:col + 1], scalar2=0.0,
                        op0=ALU.add, op1=ALU.max)
                Y2 = sb.tile([P, BATCH * QPE], F16, name="y2", tag="y2", bufs=2)
                nc.vector.tensor_scalar(
                    out=Y2[:, 0:ncols * QPE], in0=Y[:, 0:ncols * QPE],
                    scalar1=1.0, scalar2=None, op0=ALU.min)
                for k, col in enumerate(cols):
                    for m in range(QPE // MMN):
                        nc.tensor.matmul(
                            PSB[0:1, m * MMN:(m + 1) * MMN],
                            lhsT=ONES16[:, 0:1],
                            rhs=Y2[:, k * QPE + m * MMN:k * QPE + (m + 1) * MMN],
                            start=(col == 0), stop=(col == NCOL - 1))

            # fg part (a_i - 0.5 for all 2048 queries) on the same path
            YA = sb.tile([P, FG], F16)
            for col in range(FGS // P):
                nc.vector.tensor_scalar(
                    out=YA[:], in0=CQF[:], scalar1=UF32[:, col:col + 1],
                    scalar2=0.0, op0=ALU.add, op1=ALU.max)
                YA2 = sb.tile([P, FG], F16, name="ya2", tag="ya2", bufs=2)
                nc.vector.tensor_scalar(
                    out=YA2[:], in0=YA[:], scalar1=1.0, scalar2=None,
                    op0=ALU.min)
                for m in range(FG // MMN):
                    nc.tensor.matmul(
                        PSA[0:1, m * MMN:(m + 1) * MMN],
                        lhsT=ONES16[:, 0:1],
                        rhs=YA2[:, m * MMN:(m + 1) * MMN],
                        start=(col == 0), stop=(col == FGS // P - 1))

            # ---------------- ScalarE path: blocks 12..15 ----------------
            ACC0 = sb.tile([P, QB], F32)
            ACC1 = sb.tile([P, QB], F32)
            SCRS = sb.tile([P, BGS // 2], F16)
            acs = [(ACC0, c0), (ACC1, c1)]
            # half-data sub-accumulators to keep the act scratch at 32KB/partition
            ACC0b = sb.tile([P, QB], F32)
            ACC1b = sb.tile([P, QB], F32)
            acsb = [(ACC0b, c0), (ACC1b, c1)]
            H = BGS // 2
            for blk in range(NBPE, QB):
                for (acc, cc), (accb, _) in zip(acs, acsb):
                    nc.scalar.activation(
                        out=SCRS[:], in_=UBC[:, 0:H], func=AF.Relu,
                        bias=cc[:, blk:blk + 1], scale=1.0,
                        accum_out=acc[:, blk:blk + 1])
                    nc.scalar.activation(
                        out=SCRS[:], in_=UBC[:, H:BGS], func=AF.Relu,
                        bias=cc[:, blk:blk + 1], scale=1.0,
                        accum_out=accb[:, blk:blk + 1])

            # ---------------- merge partials, AllReduce ----------------
            # PE PSUM rows -> SBUF rows -> reshape via SBUF->SBUF DMA
            BROW = sb.tile([1, QPE], F32)
            nc.vector.tensor_copy(BROW[:], PSB[0:1, :])
            AROW = sb.tile([1, FG], F32)
            nc.vector.tensor_copy(AROW[:], PSA[0:1, :])
            PEB = sb.tile([P, NBPE], F32)
            nc.sync.dma_start(out=PEB[:], in_=BROW[0:1, :])
            AFG = sb.tile([P, QB], F32)
            nc.sync.dma_start(out=AFG[:], in_=AROW[0:1, :])

            CMB = sb.tile([P, QB], F32)
            nc.vector.tensor_copy(CMB[:, 0:NBPE], PEB[:])
            # ScalarE blocks: b = (acc0 + acc0b) - (acc1 + acc1b)
            S0 = sb.tile([P, QB - NBPE], F32)
            nc.vector.tensor_add(S0[:], ACC0[:, NBPE:QB], ACC0b[:, NBPE:QB])
            S1 = sb.tile([P, QB - NBPE], F32)
            nc.vector.tensor_add(S1[:], ACC1[:, NBPE:QB], ACC1b[:, NBPE:QB])
            nc.vector.tensor_sub(CMB[:, NBPE:QB], S0[:], S1[:])

            cc_in = dram.tile([2 * FG], F32)
            cc_out = dram.tile([2 * FG], F32)
            nc.sync.dma_start(out=cc_in[0:FG].rearrange("(p c) -> p c", p=P),
                              in_=CMB[:])
            nc.sync.dma_start(out=cc_in[FG:2 * FG].rearrange("(p c) -> p c", p=P),
                              in_=AFG[:])
            nc.gpsimd.collective_compute(
                "AllReduce", ALU.add,
                replica_groups=[list(range(NCORES))],
                ins=[cc_in.opt()], outs=[cc_out.opt()])
            BT = sb.tile([P, QB], F32)
            nc.sync.dma_start(out=BT[:],
                              in_=cc_out[0:FG].rearrange("(p c) -> p c", p=P))
            AT = sb.tile([P, QB], F32)
            nc.sync.dma_start(out=AT[:],
                              in_=cc_out[FG:2 * FG].rearrange("(p c) -> p c", p=P))

            # ---------------- cur = a / (a + b) ----------------
            Aq = sb.tile([P, QB], F32)
            nc.vector.tensor_scalar(out=Aq[:], in0=AT[:], scalar1=0.5,
                                    scalar2=None, op0=ALU.add)
            Sq = sb.tile([P, QB], F32)
            nc.vector.tensor_add(Sq[:], Aq[:], BT[:])
            RS = sb.tile([P, QB], F32)
            nc.vector.reciprocal(RS[:], Sq[:])
            CUR = sb.tile([P, QB], F32)
            nc.vector.tensor_tensor(out=CUR[:], in0=Aq[:], in1=RS[:], op=ALU.mult)

            # ---------------- prec = masked running max ----------------
            VQH = sb.tile([P, QB], F16)       # fp16-rounded queries: self-compare safe
            nc.vector.tensor_copy(VQH[:], F[:])
            VQR = sb.tile([P, QB], F32)
            nc.vector.tensor_copy(VQR[:], VQH[:])
            CUR16 = sb.tile([P, QB], F16)
            nc.vector.tensor_copy(CUR16[:], CUR[:])

            vline = dram.tile([FG], F16)
            nc.sync.dma_start(out=vline[:].rearrange("(p c) -> p c", p=P),
                              in_=VQH[:])
            cline = dram.tile([FG], F16)
            nc.sync.dma_start(out=cline[:].rearrange("(p c) -> p c", p=P),
                              in_=CUR16[:])
            VB = sb.tile([P, FG], F16)
            nc.sync.dma_start(out=VB[:],
                              in_=vline[:].unsqueeze(0).broadcast_to([P, FG]))
            CB = sb.tile([P, FG], F16)
            nc.sync.dma_start(out=CB[:],
                              in_=cline[:].unsqueeze(0).broadcast_to([P, FG]))

            PREC = sb.tile([P, QB], F32)
            MSK = sb.tile([P, FG], F16)
            TM = sb.tile([P, FG], F16)
            for blk in range(QB):
                nc.vector.tensor_scalar(out=MSK[:], in0=VB[:],
                                        scalar1=VQR[:, blk:blk + 1],
                                        scalar2=-10000.0,
                                        op0=ALU.is_gt, op1=ALU.mult)
                nc.vector.tensor_tensor(out=TM[:], in0=MSK[:], in1=CB[:],
                                        op=ALU.add)
                nc.vector.tensor_reduce(out=PREC[:, blk:blk + 1], in_=TM[:],
                                        axis=mybir.AxisListType.X, op=ALU.max)

            # ---------------- metric = 1 - sum(prec)/FG ----------------
            PSUM_ = sb.tile([P, 1], F32)
            nc.vector.tensor_reduce(out=PSUM_[:], in_=PREC[:],
                                    axis=mybir.AxisListType.X, op=ALU.add)
            ONESF = sb.tile([P, 1], F32)
            nc.vector.memset(ONESF[:], 1.0)
            TOT = ps.tile([1, 1], F32)
            nc.tensor.matmul(TOT[:], lhsT=PSUM_[:, 0:1], rhs=ONESF[:, 0:1],
                             start=True, stop=True)
            MT = sb.tile([1, 1], F32)
            nc.vector.tensor_scalar(out=MT[:], in0=TOT[0:1, 0:1],
                                    scalar1=-1.0 / FG, scalar2=1.0,
                                    op0=ALU.mult, op1=ALU.add)
            nc.sync.dma_start(out=out_e[:, :], in_=MT[:])
    nc.compile()
    return nc


def _get_compiled():
    global _compiled
    if _compiled is None:
        _compiled = _build()
    return _compiled


def kernel(logits, targets, _trace=False, _trace_kwargs=None):
    from concourse.bass_utils import run_bass_kernel_spmd

    logits = np.ascontiguousarray(np.asarray(logits), dtype=np.float32)
    targets = np.ascontiguousarray(np.asarray(targets), dtype=np.int32)
    fg = logits[:FG]
    bg = logits[FG:]
    tg = targets[FG:]
    in_maps = []
    for r in range(NCORES):
        in_maps.append({
            "fgq": fg,
            "fgsh": np.ascontiguousarray(fg[r * FGS:(r + 1) * FGS]),
            "bgsh": np.ascontiguousarray(bg[r * BGS:(r + 1) * BGS]),
            "tgt": np.ascontiguousarray(tg[r * BGS:(r + 1) * BGS]),
        })
    nc = _get_compiled()
    kw = {}
    if _trace:
        kw = dict(trace=True, **(_trace_kwargs or {}))
    res = run_bass_kernel_spmd(nc, in_maps, core_ids=list(range(NCORES)), **kw)
    out = np.float32(res.results[0]["out"][0, 0])
    # metric = 1 - mean(prec) with prec in (0,1] is always in [0,1); an
    # out-of-range value means the device was left in a bad state by a
    # previously killed run -- retry once on a clean execution.
    if not (-1e-3 <= float(out) <= 1.0 + 1e-3):
        res = run_bass_kernel_spmd(nc, in_maps, core_ids=list(range(NCORES)), **kw)
        out = np.float32(res.results[0]["out"][0, 0])
    if _trace:
        return out, res
    return out


if __name__ == "__main__":
    rng = np.random.default_rng(0)
    logits = rng.standard_normal(N).astype(np.float32)
    targets = np.concatenate([np.ones(FG, np.int32), np.zeros(BG, np.int32)])
    print("metric:", kernel(logits, targets))


# revision 13
# speedup vs baseline: 2.6423x; 1.0223x over previous
"""Trainium2 Bass kernel for the AP-loss metric (nn_APLoss).

For N=262144 logits with the first FG=2048 being positives:
    metric = 1 - mean_i(prec_i),  prec_i = max{cur_j : v_j <= v_i}
    cur_i = a_i / (a_i + b_i)
    a_i = 0.5 + sum_fg clip((fg - v_i)/2 + .5, 0, 1)
    b_i = sum_{bg: target==0} clip((bg - v_i)/2 + .5, 0, 1)
(The reference's sorted scan + cummax is order-free; its bg>=min(fg)-1
threshold mask is provably redundant for the sums.)

Per core (8-way shard of bg and of the fg-data):
  clip(u + c) with u = x/2, c_i = 0.5 - v_i/2 is evaluated 66.6M times via
  three engine paths:
  - PE path (query blocks 0..11): data kept natural [128 x 254]; per data
    column a 4x-mode fp16 tensor_scalar relu pass [128 x 1536] against a
    query-broadcast tile, min-with-1 passes batched over 8 columns, then
    TensorE sums along partitions with accumulating matmul chains into
    PSUM (direct clip sums).
  - ScalarE path (query blocks 12..15): activation(Relu, bias=c, accum_out)
    over partition-broadcast data, clip sum = relu-sum(c0) - relu-sum(c1).
  - The fg-data part (a_i, all 2048 queries x 256 elems) rides the PE path.
  One 16KB AllReduce of per-query partials; postprocessing (cur, masked
  running max, mean) replicated on every core.
"""

import os
import sys

import numpy as np

sys.path.insert(0, "/opt/trn_rl_repo")

P = 128
FG = 2048
N = 262144
BG = N - FG
NCORES = 8
QB = FG // P            # 16 query blocks
FGS = FG // NCORES      # 256 fg data elems per core
BGS = BG // NCORES      # 32512 bg elems per core
NCOL = BGS // P         # 254 data columns per core
QPE = 1536              # queries on the PE path (blocks 0..11)
NBPE = QPE // P         # 12
MMN = 512               # matmul moving-dim chunk (one PSUM bank)
BATCH = 8               # data columns per min-pass batch
# min-pass batches routed to ScalarE as z=relu(1-y) (summed with -ones,
# plus a count constant); the rest stay on VectorE as in-place min(y,1).
SBATCHES = (3, 9, 15, 21, 27)

_compiled = None


def _build():
    import concourse.bacc as bacc
    import concourse.tile as tile
    from concourse import mybir

    F32 = mybir.dt.float32
    F16 = mybir.dt.float16
    I32 = mybir.dt.int32
    ALU = mybir.AluOpType
    AF = mybir.ActivationFunctionType

    nc = bacc.Bacc("TRN2", target_bir_lowering=False, debug=False,
                   num_devices=NCORES)
    fgq_e = nc.declare_dram_parameter("fgq", [FG], F32, isOutput=False)
    fgsh_e = nc.declare_dram_parameter("fgsh", [FGS], F32, isOutput=False)
    bgsh_e = nc.declare_dram_parameter("bgsh", [BGS], F32, isOutput=False)
    tgt_e = nc.declare_dram_parameter("tgt", [BGS], I32, isOutput=False)
    prq_e = nc.declare_dram_parameter("prq", [2 * P], F32, isOutput=False)
    out_e = nc.declare_dram_parameter("out", [1, 1], F32, isOutput=True)

    with tile.TileContext(nc) as tc:
        with tc.tile_pool(name="sbuf", bufs=1) as sb, \
             tc.tile_pool(name="dram", bufs=1, space="DRAM") as dram, \
             tc.tile_pool(name="psum", bufs=1, space="PSUM") as ps:

            # ---------------- inputs ----------------
            F = sb.tile([P, QB], F32)         # all 2048 fg; query i at (p, blk)=p*QB+blk
            nc.sync.dma_start(out=F[:], in_=fgq_e[:].rearrange("(p c) -> p c", p=P))
            FS = sb.tile([P, FGS // P], F32)  # this core's fg data shard
            nc.sync.dma_start(out=FS[:], in_=fgsh_e[:].rearrange("(p c) -> p c", p=P))
            B = sb.tile([P, NCOL], F32)
            nc.sync.dma_start(out=B[:], in_=bgsh_e[:].rearrange("(p c) -> p c", p=P))
            T = sb.tile([P, NCOL], I32)
            nc.sync.dma_start(out=T[:], in_=tgt_e[:].rearrange("(p c) -> p c", p=P))

            # ---------------- query tiles ----------------
            c0 = sb.tile([P, QB], F32)        # c0 = 0.5 - 0.5 v
            nc.vector.tensor_scalar(out=c0[:], in0=F[:], scalar1=-0.5,
                                    scalar2=0.5, op0=ALU.mult, op1=ALU.add)
            c1 = sb.tile([P, QB], F32)        # c1 = c0 - 1
            nc.vector.tensor_scalar(out=c1[:], in0=c0[:], scalar1=-1.0,
                                    scalar2=None, op0=ALU.add)
            c0h = sb.tile([P, QB], F16)
            nc.vector.tensor_copy(c0h[:], c0[:])

            # query-broadcast lines for the PE path
            cqpe_d = dram.tile([QPE], F16)    # c0 of blocks 0..11, (p,blk) order
            nc.sync.dma_start(out=cqpe_d[:].rearrange("(p c) -> p c", p=P),
                              in_=c0h[:, 0:NBPE])
            cqf_d = dram.tile([FG], F16)      # c0 of all blocks (for the fg part)
            nc.sync.dma_start(out=cqf_d[:].rearrange("(p c) -> p c", p=P),
                              in_=c0h[:])
            CQPE = sb.tile([P, QPE], F16)
            nc.sync.dma_start(out=CQPE[:],
                              in_=cqpe_d[:].unsqueeze(0).broadcast_to([P, QPE]))
            CQF = sb.tile([P, FG], F16)
            nc.sync.dma_start(out=CQF[:],
                              in_=cqf_d[:].unsqueeze(0).broadcast_to([P, FG]))

            # ---------------- data preprocess ----------------
            # u_bg = target==0 ? 0.5*bg : -500 (-500 -> clip contributes 0)
            M = sb.tile([P, NCOL], F32)
            nc.vector.tensor_scalar(out=M[:], in0=T[:], scalar1=0,
                                    scalar2=None, op0=ALU.is_equal)
            T1 = sb.tile([P, NCOL], F32)
            nc.vector.tensor_scalar(out=T1[:], in0=B[:], scalar1=0.5,
                                    scalar2=500.0, op0=ALU.mult, op1=ALU.add)
            T2 = sb.tile([P, NCOL], F32)
            nc.vector.tensor_tensor(out=T2[:], in0=T1[:], in1=M[:], op=ALU.mult)
            U32 = sb.tile([P, NCOL], F32)     # natural-layout data (PE-path scalars)
            nc.vector.tensor_scalar(out=U32[:], in0=T2[:], scalar1=-500.0,
                                    scalar2=None, op0=ALU.add)
            UF32 = sb.tile([P, FGS // P], F32)
            nc.vector.tensor_scalar(out=UF32[:], in0=FS[:], scalar1=0.5,
                                    scalar2=None, op0=ALU.mult)

            # fp16 copy -> HBM -> partition-broadcast (ScalarE path data)
            U16 = sb.tile([P, NCOL], F16)
            nc.vector.tensor_copy(U16[:], U32[:])
            udata = dram.tile([BGS], F16)
            H = BGS // 2
            nc.sync.dma_start(out=udata[0:H].rearrange("(p c) -> p c", p=P),
                              in_=U16[:, 0:NCOL // 2])
            nc.sync.dma_start(out=udata[H:BGS].rearrange("(p c) -> p c", p=P),
                              in_=U16[:, NCOL // 2:NCOL])
            UBC = sb.tile([P, BGS], F16)
            nc.sync.dma_start(out=UBC[:, 0:H],
                              in_=udata[0:H].unsqueeze(0).broadcast_to([P, H]))
            nc.sync.dma_start(out=UBC[:, H:BGS],
                              in_=udata[H:BGS].unsqueeze(0).broadcast_to([P, H]))

            # ---------------- PE path: bg clip sums for blocks 0..11 ----------------
            ONES16 = sb.tile([P, 1], F16)
            nc.vector.memset(ONES16[:], 1.0)
            NEG16 = sb.tile([P, 1], F16)
            nc.vector.memset(NEG16[:], -1.0)
            PSB = ps.tile([1, QPE], F32)      # 3 banks
            PSA = ps.tile([1, FG], F32)       # 4 banks
            nbatch = (NCOL + BATCH - 1) // BATCH
            for bt in range(nbatch):
                cols = range(bt * BATCH, min((bt + 1) * BATCH, NCOL))
                ncols = len(cols)
                on_s = bt in SBATCHES
                Y2 = sb.tile([P, BATCH * QPE], F16, name="y2", tag="y2", bufs=2)
                for k, col in enumerate(cols):
                    nc.vector.tensor_scalar(
                        out=Y2[:, k * QPE:(k + 1) * QPE], in0=CQPE[:],
                        scalar1=U32[:, col:col + 1], scalar2=0.0,
                        op0=ALU.add, op1=ALU.max)
                if on_s:
                    # z = relu(1 - y); sum(min(y,1)) = count - sum(z)
                    nc.scalar.activation(
                        out=Y2[:, 0:ncols * QPE], in_=Y2[:, 0:ncols * QPE],
                        func=AF.Relu, bias=1.0, scale=-1.0)
                else:
                    nc.vector.tensor_scalar(
                        out=Y2[:, 0:ncols * QPE], in0=Y2[:, 0:ncols * QPE],
                        scalar1=1.0, scalar2=None, op0=ALU.min)
                w = NEG16 if on_s else ONES16
                for k, col in enumerate(cols):
                    for m in range(QPE // MMN):
                        nc.tensor.matmul(
                            PSB[0:1, m * MMN:(m + 1) * MMN],
                            lhsT=w[:, 0:1],
                            rhs=Y2[:, k * QPE + m * MMN:k * QPE + (m + 1) * MMN],
                            start=(col == 0), stop=(col == NCOL - 1))

            # fg part (a_i - 0.5 for all 2048 queries) on the same path
            YA = sb.tile([P, FG], F16)
            for col in range(FGS // P):
                nc.vector.tensor_scalar(
                    out=YA[:], in0=CQF[:], scalar1=UF32[:, col:col + 1],
                    scalar2=0.0, op0=ALU.add, op1=ALU.max)
                YA2 = sb.tile([P, FG], F16, name="ya2", tag="ya2", bufs=2)
                nc.vector.tensor_scalar(
                    out=YA2[:], in0=YA[:], scalar1=1.0, scalar2=None,
                    op0=ALU.min)
                for m in range(FG // MMN):
                    nc.tensor.matmul(
                        PSA[0:1, m * MMN:(m + 1) * MMN],
                        lhsT=ONES16[:, 0:1],
                        rhs=YA2[:, m * MMN:(m + 1) * MMN],
                        start=(col == 0), stop=(col == FGS // P - 1))

            # ---------------- ScalarE path: blocks 12..15 ----------------
            ACC0 = sb.tile([P, QB], F32)
            ACC1 = sb.tile([P, QB], F32)
            SCRS = sb.tile([P, BGS // 2], F16)
            acs = [(ACC0, c0), (ACC1, c1)]
            # half-data sub-accumulators to keep the act scratch at 32KB/partition
            ACC0b = sb.tile([P, QB], F32)
            ACC1b = sb.tile([P, QB], F32)
            acsb = [(ACC0b, c0), (ACC1b, c1)]
            H = BGS // 2
            for blk in range(NBPE, QB):
                for (acc, cc), (accb, _) in zip(acs, acsb):
                    nc.scalar.activation(
                        out=SCRS[:], in_=UBC[:, 0:H], func=AF.Relu,
                        bias=cc[:, blk:blk + 1], scale=1.0,
                        accum_out=acc[:, blk:blk + 1])
                    nc.scalar.activation(
                        out=SCRS[:], in_=UBC[:, H:BGS], func=AF.Relu,
                        bias=cc[:, blk:blk + 1], scale=1.0,
                        accum_out=accb[:, blk:blk + 1])

            # ---------------- merge partials, AllReduce ----------------
            # PE PSUM rows -> SBUF rows -> reshape via SBUF->SBUF DMA
            BROW = sb.tile([1, QPE], F32)
            nc.vector.tensor_copy(BROW[:], PSB[0:1, :])
            AROW = sb.tile([1, FG], F32)
            nc.vector.tensor_copy(AROW[:], PSA[0:1, :])
            PEB = sb.tile([P, NBPE], F32)
            nc.sync.dma_start(out=PEB[:], in_=BROW[0:1, :])
            AFG = sb.tile([P, QB], F32)
            nc.sync.dma_start(out=AFG[:], in_=AROW[0:1, :])

            CMB = sb.tile([P, QB], F32)
            nc.vector.tensor_copy(CMB[:, 0:NBPE], PEB[:])
            # ScalarE blocks: b = (acc0 + acc0b) - (acc1 + acc1b)
            S0 = sb.tile([P, QB - NBPE], F32)
            nc.vector.tensor_add(S0[:], ACC0[:, NBPE:QB], ACC0b[:, NBPE:QB])
            S1 = sb.tile([P, QB - NBPE], F32)
            nc.vector.tensor_add(S1[:], ACC1[:, NBPE:QB], ACC1b[:, NBPE:QB])
            nc.vector.tensor_sub(CMB[:, NBPE:QB], S0[:], S1[:])

            cc_in = dram.tile([2 * FG], F32)
            cc_out = dram.tile([2 * FG], F32)
            nc.sync.dma_start(out=cc_in[0:FG].rearrange("(p c) -> p c", p=P),
                              in_=CMB[:])
            nc.sync.dma_start(out=cc_in[FG:2 * FG].rearrange("(p c) -> p c", p=P),
                              in_=AFG[:])
            nc.gpsimd.collective_compute(
                "AllReduce", ALU.add,
                replica_groups=[list(range(NCORES))],
                ins=[cc_in.opt()], outs=[cc_out.opt()])
            BT = sb.tile([P, QB], F32)
            nc.sync.dma_start(out=BT[:],
                              in_=cc_out[0:FG].rearrange("(p c) -> p c", p=P))
            AT = sb.tile([P, QB], F32)
            nc.sync.dma_start(out=AT[:],
                              in_=cc_out[FG:2 * FG].rearrange("(p c) -> p c", p=P))
            # count constant for the ScalarE-min batches (z-form)
            zcols = sum(min((bt + 1) * BATCH, NCOL) - bt * BATCH for bt in SBATCHES)
            nc.vector.tensor_scalar(out=BT[:, 0:NBPE], in0=BT[:, 0:NBPE],
                                    scalar1=float(NCORES * P * zcols),
                                    scalar2=None, op0=ALU.add)

            # ---------------- cur = a / (a + b) ----------------
            Aq = sb.tile([P, QB], F32)
            nc.vector.tensor_scalar(out=Aq[:], in0=AT[:], scalar1=0.5,
                                    scalar2=None, op0=ALU.add)
            Sq = sb.tile([P, QB], F32)
            nc.vector.tensor_add(Sq[:], Aq[:], BT[:])
            RS = sb.tile([P, QB], F32)
            nc.vector.reciprocal(RS[:], Sq[:])
            CUR = sb.tile([P, QB], F32)
            nc.vector.tensor_tensor(out=CUR[:], in0=Aq[:], in1=RS[:], op=ALU.mult)

            # ------- prec = masked running max (2 query blocks per core) -------
            VQH = sb.tile([P, QB], F16)       # fp16-rounded queries: self-compare safe
            nc.vector.tensor_copy(VQH[:], F[:])
            CUR16 = sb.tile([P, QB], F16)
            nc.vector.tensor_copy(CUR16[:], CUR[:])

            vline = dram.tile([FG], F16)
            nc.sync.dma_start(out=vline[:].rearrange("(p c) -> p c", p=P),
                              in_=VQH[:])
            cline = dram.tile([FG], F16)
            nc.sync.dma_start(out=cline[:].rearrange("(p c) -> p c", p=P),
                              in_=CUR16[:])
            VB = sb.tile([P, FG], F16)
            nc.sync.dma_start(out=VB[:],
                              in_=vline[:].unsqueeze(0).broadcast_to([P, FG]))
            CB = sb.tile([P, FG], F16)
            nc.sync.dma_start(out=CB[:],
                              in_=cline[:].unsqueeze(0).broadcast_to([P, FG]))

            # this core's 2 query blocks (input-sharded), fp16-rounded like VB
            PRQ = sb.tile([P, 2], F32)
            nc.sync.dma_start(out=PRQ[:], in_=prq_e[:].rearrange("(c p) -> p c", p=P))
            PRQH = sb.tile([P, 2], F16)
            nc.vector.tensor_copy(PRQH[:], PRQ[:])
            PRQR = sb.tile([P, 2], F32)
            nc.vector.tensor_copy(PRQR[:], PRQH[:])

            PREC = sb.tile([P, 2], F32)
            MSK = sb.tile([P, FG], F16)
            TM = sb.tile([P, FG], F16)
            for blk in range(2):
                nc.vector.tensor_scalar(out=MSK[:], in0=VB[:],
                                        scalar1=PRQR[:, blk:blk + 1],
                                        scalar2=-10000.0,
                                        op0=ALU.is_gt, op1=ALU.mult)
                nc.vector.tensor_tensor(out=TM[:], in0=MSK[:], in1=CB[:],
                                        op=ALU.add)
                nc.vector.tensor_reduce(out=PREC[:, blk:blk + 1], in_=TM[:],
                                        axis=mybir.AxisListType.X, op=ALU.max)

            # ---------------- metric = 1 - sum(prec)/FG ----------------
            PSUM_ = sb.tile([P, 1], F32)
            nc.vector.tensor_reduce(out=PSUM_[:], in_=PREC[:],
                                    axis=mybir.AxisListType.X, op=ALU.add)
            ONESF = sb.tile([P, 1], F32)
            nc.vector.memset(ONESF[:], 1.0)
            TOT = ps.tile([1, 1], F32)
            nc.tensor.matmul(TOT[:], lhsT=PSUM_[:, 0:1], rhs=ONESF[:, 0:1],
                             start=True, stop=True)
            PP = sb.tile([1, 1], F32)
            nc.vector.tensor_copy(PP[:], TOT[0:1, 0:1])
            cc2_in = dram.tile([1], F32)
            cc2_out = dram.tile([1], F32)
            nc.sync.dma_start(out=cc2_in[:], in_=PP[0:1, 0:1])
            nc.gpsimd.collective_compute(
                "AllReduce", ALU.add,
                replica_groups=[list(range(NCORES))],
                ins=[cc2_in.opt()], outs=[cc2_out.opt()])
            PT = sb.tile([1, 1], F32)
            nc.sync.dma_start(out=PT[0:1, 0:1], in_=cc2_out[:])
            MT = sb.tile([1, 1], F32)
            nc.vector.tensor_scalar(out=MT[:], in0=PT[0:1, 0:1],
                                    scalar1=-1.0 / FG, scalar2=1.0,
                                    op0=ALU.mult, op1=ALU.add)
            nc.sync.dma_start(out=out_e[:, :], in_=MT[:])
    nc.compile()
    return nc


def _get_compiled():
    global _compiled
    if _compiled is None:
        _compiled = _build()
    return _compiled


def kernel(logits, targets, _trace=False, _trace_kwargs=None):
    from concourse.bass_utils import run_bass_kernel_spmd

    logits = np.ascontiguousarray(np.asarray(logits), dtype=np.float32)
    targets = np.ascontiguousarray(np.asarray(targets), dtype=np.int32)
    fg = logits[:FG]
    bg = logits[FG:]
    tg = targets[FG:]
    fg_mat = fg.reshape(P, QB)   # query i at (p, blk) = p*QB + blk
    in_maps = []
    for r in range(NCORES):
        prq = np.concatenate([fg_mat[:, 2 * r], fg_mat[:, 2 * r + 1]])
        in_maps.append({
            "fgq": fg,
            "fgsh": np.ascontiguousarray(fg[r * FGS:(r + 1) * FGS]),
            "bgsh": np.ascontiguousarray(bg[r * BGS:(r + 1) * BGS]),
            "tgt": np.ascontiguousarray(tg[r * BGS:(r + 1) * BGS]),
            "prq": np.ascontiguousarray(prq),
        })
    nc = _get_compiled()
    kw = {}
    if _trace:
        kw = dict(trace=True, **(_trace_kwargs or {}))
    res = run_bass_kernel_spmd(nc, in_maps, core_ids=list(range(NCORES)), **kw)
    out = np.float32(res.results[0]["out"][0, 0])
    # metric = 1 - mean(prec) with prec in (0,1] is always in [0,1); an
    # out-of-range value means the device was left in a bad state by a
    # previously killed run -- retry once on a clean execution.
    if not (-1e-3 <= float(out) <= 1.0 + 1e-3):
        res = run_bass_kernel_spmd(nc, in_maps, core_ids=list(range(NCORES)), **kw)
        out = np.float32(res.results[0]["out"][0, 0])
    if _trace:
        return out, res
    return out


if __name__ == "__main__":
    rng = np.random.default_rng(0)
    logits = rng.standard_normal(N).astype(np.float32)
    targets = np.concatenate([np.ones(FG, np.int32), np.zeros(BG, np.int32)])
    print("metric:", kernel(logits, targets))


# revision 14
# speedup vs baseline: 3.2732x; 1.2388x over previous
"""Trainium2 Bass kernel for the AP-loss metric (nn_APLoss).

For N=262144 logits with the first FG=2048 being positives:
    metric = 1 - mean_i(prec_i),  prec_i = max{cur_j : v_j <= v_i}
    cur_i = a_i / (a_i + b_i)
    a_i = 0.5 + sum_fg clip((fg - v_i)/2 + .5, 0, 1)
    b_i = sum_{bg: target==0} clip((bg - v_i)/2 + .5, 0, 1)
(The reference's sorted scan + cummax is order-free; its bg>=min(fg)-1
threshold mask is provably redundant for the sums.)

Core identity: with u = x/2 and c_i = 0.5 - v_i/2,
    clip(c_i + u_j, 0, 1) = max(min(c_i, 1-u_j), -u_j) + u_j
so one fp16 4x-mode tensor_scalar per data column (queries broadcast along
the free axis, 1-u / -u as per-partition scalars) yields clip values minus
a u_j term whose sum is a per-query constant (sum_j u_j), added back to the
partials before the AllReduce. Invalid bg (target!=0) pinned to u=-500
still contributes exactly 0. TensorE sums the clip tiles along partitions
with accumulating matmul chains into PSUM (one 512-wide chain per PSUM
bank). bg and fg-data are 8-way sharded; one 16KB AllReduce combines
per-query partials; cur and the masked running max run per-core (each core
handles 2 of the 16 query blocks via the prq input), with a second scalar
AllReduce for the prec sum.
"""

import os
import sys

import numpy as np

sys.path.insert(0, "/opt/trn_rl_repo")

P = 128
FG = 2048
N = 262144
BG = N - FG
NCORES = 8
QB = FG // P            # 16 query blocks
FGS = FG // NCORES      # 256 fg data elems per core
BGS = BG // NCORES      # 32512 bg elems per core
NCOL = BGS // P         # 254 data columns per core
MMN = 512               # matmul moving-dim chunk (one PSUM bank)

_compiled = None


def _build():
    import concourse.bacc as bacc
    import concourse.tile as tile
    from concourse import mybir

    F32 = mybir.dt.float32
    F16 = mybir.dt.float16
    I32 = mybir.dt.int32
    ALU = mybir.AluOpType

    nc = bacc.Bacc("TRN2", target_bir_lowering=False, debug=False,
                   num_devices=NCORES)
    fgq_e = nc.declare_dram_parameter("fgq", [FG], F32, isOutput=False)
    fgsh_e = nc.declare_dram_parameter("fgsh", [FGS], F32, isOutput=False)
    bgsh_e = nc.declare_dram_parameter("bgsh", [BGS], F32, isOutput=False)
    tgt_e = nc.declare_dram_parameter("tgt", [BGS], I32, isOutput=False)
    prq_e = nc.declare_dram_parameter("prq", [2 * P], F32, isOutput=False)
    out_e = nc.declare_dram_parameter("out", [1, 1], F32, isOutput=True)

    with tile.TileContext(nc) as tc:
        with tc.tile_pool(name="sbuf", bufs=1) as sb, \
             tc.tile_pool(name="dram", bufs=1, space="DRAM") as dram, \
             tc.tile_pool(name="psum", bufs=1, space="PSUM") as ps:

            # ---------------- inputs ----------------
            F = sb.tile([P, QB], F32)         # all 2048 fg; query i at (p, blk)=p*QB+blk
            nc.sync.dma_start(out=F[:], in_=fgq_e[:].rearrange("(p c) -> p c", p=P))
            FS = sb.tile([P, FGS // P], F32)  # this core's fg data shard
            nc.sync.dma_start(out=FS[:], in_=fgsh_e[:].rearrange("(p c) -> p c", p=P))
            B = sb.tile([P, NCOL], F32)
            nc.sync.dma_start(out=B[:], in_=bgsh_e[:].rearrange("(p c) -> p c", p=P))
            T = sb.tile([P, NCOL], I32)
            nc.sync.dma_start(out=T[:], in_=tgt_e[:].rearrange("(p c) -> p c", p=P))

            # ---------------- query broadcast tile ----------------
            c0 = sb.tile([P, QB], F32)        # c0 = 0.5 - 0.5 v
            nc.vector.tensor_scalar(out=c0[:], in0=F[:], scalar1=-0.5,
                                    scalar2=0.5, op0=ALU.mult, op1=ALU.add)
            c0h = sb.tile([P, QB], F16)
            nc.vector.tensor_copy(c0h[:], c0[:])
            cq_d = dram.tile([FG], F16)       # (p,blk) order == query index order
            nc.sync.dma_start(out=cq_d[:].rearrange("(p c) -> p c", p=P),
                              in_=c0h[:])
            CQF = sb.tile([P, FG], F16)
            nc.sync.dma_start(out=CQF[:],
                              in_=cq_d[:].unsqueeze(0).broadcast_to([P, FG]))

            # ---------------- data preprocess ----------------
            # u_bg = target==0 ? 0.5*bg : -500 (-500 -> clip contributes 0)
            M = sb.tile([P, NCOL], F32)
            nc.vector.tensor_scalar(out=M[:], in0=T[:], scalar1=0,
                                    scalar2=None, op0=ALU.is_equal)
            T1 = sb.tile([P, NCOL], F32)
            nc.vector.tensor_scalar(out=T1[:], in0=B[:], scalar1=0.5,
                                    scalar2=500.0, op0=ALU.mult, op1=ALU.add)
            T2 = sb.tile([P, NCOL], F32)
            nc.vector.tensor_tensor(out=T2[:], in0=T1[:], in1=M[:], op=ALU.mult)
            U32 = sb.tile([P, NCOL], F32)
            nc.vector.tensor_scalar(out=U32[:], in0=T2[:], scalar1=-500.0,
                                    scalar2=None, op0=ALU.add)
            UF32 = sb.tile([P, FGS // P], F32)
            nc.vector.tensor_scalar(out=UF32[:], in0=FS[:], scalar1=0.5,
                                    scalar2=None, op0=ALU.mult)
            # per-partition scalars for the fused clip op
            UM1 = sb.tile([P, NCOL], F32)     # 1 - u
            nc.vector.tensor_scalar(out=UM1[:], in0=U32[:], scalar1=-1.0,
                                    scalar2=1.0, op0=ALU.mult, op1=ALU.add)
            UNEG = sb.tile([P, NCOL], F32)    # -u
            nc.vector.tensor_scalar(out=UNEG[:], in0=U32[:], scalar1=-1.0,
                                    scalar2=None, op0=ALU.mult)
            UFM1 = sb.tile([P, FGS // P], F32)
            nc.vector.tensor_scalar(out=UFM1[:], in0=UF32[:], scalar1=-1.0,
                                    scalar2=1.0, op0=ALU.mult, op1=ALU.add)
            UFNEG = sb.tile([P, FGS // P], F32)
            nc.vector.tensor_scalar(out=UFNEG[:], in0=UF32[:], scalar1=-1.0,
                                    scalar2=None, op0=ALU.mult)

            # sum_j u_j correction terms (per-query constants), as [128,1]
            # broadcasts via a DRAM bounce
            SUR = sb.tile([P, 1], F32)
            nc.vector.tensor_reduce(out=SUR[:], in_=U32[:],
                                    axis=mybir.AxisListType.X, op=ALU.add)
            SUFR = sb.tile([P, 1], F32)
            nc.vector.tensor_reduce(out=SUFR[:], in_=UF32[:],
                                    axis=mybir.AxisListType.X, op=ALU.add)
            SURT = sb.tile([1, P], F32)
            nc.sync.dma_start(out=SURT[0:1, :], in_=SUR[:, 0:1])
            SUFRT = sb.tile([1, P], F32)
            nc.sync.dma_start(out=SUFRT[0:1, :], in_=SUFR[:, 0:1])
            SUT = sb.tile([1, 1], F32)
            nc.vector.tensor_reduce(out=SUT[0:1, 0:1], in_=SURT[0:1, :],
                                    axis=mybir.AxisListType.X, op=ALU.add)
            SUFT = sb.tile([1, 1], F32)
            nc.vector.tensor_reduce(out=SUFT[0:1, 0:1], in_=SUFRT[0:1, :],
                                    axis=mybir.AxisListType.X, op=ALU.add)
            su_d = dram.tile([1], F32)
            nc.sync.dma_start(out=su_d[:], in_=SUT[0:1, 0:1])
            suf_d = dram.tile([1], F32)
            nc.sync.dma_start(out=suf_d[:], in_=SUFT[0:1, 0:1])
            SUB = sb.tile([P, 1], F32)
            nc.sync.dma_start(out=SUB[:],
                              in_=su_d[:].unsqueeze(0).broadcast_to([P, 1]))
            SUFB = sb.tile([P, 1], F32)
            nc.sync.dma_start(out=SUFB[:],
                              in_=suf_d[:].unsqueeze(0).broadcast_to([P, 1]))

            # ---------------- PE-summed clip tiles ----------------
            ONES16 = sb.tile([P, 1], F16)
            nc.vector.memset(ONES16[:], 1.0)
            PSA = ps.tile([1, FG], F32)       # 4 banks: fg part (a - 0.5)
            PSB = ps.tile([1, FG], F32)       # 4 banks: bg part (b)

            for col in range(FGS // P):       # fg part first (finishes early)
                YA = sb.tile([P, FG], F16, name="ya", tag="ya", bufs=2)
                nc.vector.tensor_scalar(
                    out=YA[:], in0=CQF[:], scalar1=UFM1[:, col:col + 1],
                    scalar2=UFNEG[:, col:col + 1], op0=ALU.min, op1=ALU.max)
                for m in range(FG // MMN):
                    nc.tensor.matmul(
                        PSA[0:1, m * MMN:(m + 1) * MMN],
                        lhsT=ONES16[:, 0:1],
                        rhs=YA[:, m * MMN:(m + 1) * MMN],
                        start=(col == 0), stop=(col == FGS // P - 1))
            AROW = sb.tile([1, FG], F32)
            nc.vector.tensor_copy(AROW[:], PSA[0:1, :])

            for col in range(NCOL):
                Y2 = sb.tile([P, FG], F16, name="y2", tag="y2", bufs=4)
                nc.vector.tensor_scalar(
                    out=Y2[:], in0=CQF[:], scalar1=UM1[:, col:col + 1],
                    scalar2=UNEG[:, col:col + 1], op0=ALU.min, op1=ALU.max)
                for m in range(FG // MMN):
                    nc.tensor.matmul(
                        PSB[0:1, m * MMN:(m + 1) * MMN],
                        lhsT=ONES16[:, 0:1],
                        rhs=Y2[:, m * MMN:(m + 1) * MMN],
                        start=(col == 0), stop=(col == NCOL - 1))
            BROW = sb.tile([1, FG], F32)
            nc.vector.tensor_copy(BROW[:], PSB[0:1, :])

            # ---------------- merge + corrections + AllReduce ----------------
            PEB = sb.tile([P, QB], F32)
            nc.sync.dma_start(out=PEB[:], in_=BROW[0:1, :])
            AFG = sb.tile([P, QB], F32)
            nc.sync.dma_start(out=AFG[:], in_=AROW[0:1, :])
            PEBc = sb.tile([P, QB], F32)
            nc.vector.tensor_scalar(out=PEBc[:], in0=PEB[:],
                                    scalar1=SUB[:, 0:1], scalar2=None,
                                    op0=ALU.add)
            AFGc = sb.tile([P, QB], F32)
            nc.vector.tensor_scalar(out=AFGc[:], in0=AFG[:],
                                    scalar1=SUFB[:, 0:1], scalar2=None,
                                    op0=ALU.add)

            cc_in = dram.tile([2 * FG], F32)
            cc_out = dram.tile([2 * FG], F32)
            nc.sync.dma_start(out=cc_in[0:FG].rearrange("(p c) -> p c", p=P),
                              in_=PEBc[:])
            nc.sync.dma_start(out=cc_in[FG:2 * FG].rearrange("(p c) -> p c", p=P),
                              in_=AFGc[:])
            nc.gpsimd.collective_compute(
                "AllReduce", ALU.add,
                replica_groups=[list(range(NCORES))],
                ins=[cc_in.opt()], outs=[cc_out.opt()])
            BT = sb.tile([P, QB], F32)
            nc.sync.dma_start(out=BT[:],
                              in_=cc_out[0:FG].rearrange("(p c) -> p c", p=P))
            AT = sb.tile([P, QB], F32)
            nc.sync.dma_start(out=AT[:],
                              in_=cc_out[FG:2 * FG].rearrange("(p c) -> p c", p=P))

            # ---------------- cur = a / (a + b) ----------------
            Aq = sb.tile([P, QB], F32)
            nc.vector.tensor_scalar(out=Aq[:], in0=AT[:], scalar1=0.5,
                                    scalar2=None, op0=ALU.add)
            Sq = sb.tile([P, QB], F32)
            nc.vector.tensor_add(Sq[:], Aq[:], BT[:])
            RS = sb.tile([P, QB], F32)
            nc.vector.reciprocal(RS[:], Sq[:])
            CUR = sb.tile([P, QB], F32)
            nc.vector.tensor_tensor(out=CUR[:], in0=Aq[:], in1=RS[:], op=ALU.mult)

            # ------- prec = masked running max (2 query blocks per core) -------
            VQH = sb.tile([P, QB], F16)       # fp16-rounded queries: self-compare safe
            nc.vector.tensor_copy(VQH[:], F[:])
            CUR16 = sb.tile([P, QB], F16)
            nc.vector.tensor_copy(CUR16[:], CUR[:])

            vline = dram.tile([FG], F16)
            nc.sync.dma_start(out=vline[:].rearrange("(p c) -> p c", p=P),
                              in_=VQH[:])
            cline = dram.tile([FG], F16)
            nc.sync.dma_start(out=cline[:].rearrange("(p c) -> p c", p=P),
                              in_=CUR16[:])
            VB = sb.tile([P, FG], F16)
            nc.sync.dma_start(out=VB[:],
                              in_=vline[:].unsqueeze(0).broadcast_to([P, FG]))
            CB = sb.tile([P, FG], F16)
            nc.sync.dma_start(out=CB[:],
                              in_=cline[:].unsqueeze(0).broadcast_to([P, FG]))

            # this core's 2 query blocks (input-sharded), fp16-rounded like VB
            PRQ = sb.tile([P, 2], F32)
            nc.sync.dma_start(out=PRQ[:], in_=prq_e[:].rearrange("(c p) -> p c", p=P))
            PRQH = sb.tile([P, 2], F16)
            nc.vector.tensor_copy(PRQH[:], PRQ[:])
            PRQR = sb.tile([P, 2], F32)
            nc.vector.tensor_copy(PRQR[:], PRQH[:])

            PREC = sb.tile([P, 2], F32)
            MSK = sb.tile([P, FG], F16)
            TM = sb.tile([P, FG], F16)
            for blk in range(2):
                nc.vector.tensor_scalar(out=MSK[:], in0=VB[:],
                                        scalar1=PRQR[:, blk:blk + 1],
                                        scalar2=-10000.0,
                                        op0=ALU.is_gt, op1=ALU.mult)
                nc.vector.tensor_tensor(out=TM[:], in0=MSK[:], in1=CB[:],
                                        op=ALU.add)
                nc.vector.tensor_reduce(out=PREC[:, blk:blk + 1], in_=TM[:],
                                        axis=mybir.AxisListType.X, op=ALU.max)

            # --- metric = 1 - sum(prec)/FG  (partition sum via DMA transpose) ---
            PSUM_ = sb.tile([P, 1], F32)
            nc.vector.tensor_reduce(out=PSUM_[:], in_=PREC[:],
                                    axis=mybir.AxisListType.X, op=ALU.add)
            PSUMT = sb.tile([1, P], F32)
            nc.sync.dma_start(out=PSUMT[0:1, :], in_=PSUM_[:, 0:1])
            PP = sb.tile([1, 1], F32)
            nc.vector.tensor_reduce(out=PP[0:1, 0:1], in_=PSUMT[0:1, :],
                                    axis=mybir.AxisListType.X, op=ALU.add)
            cc2_in = dram.tile([1], F32)
            cc2_out = dram.tile([1], F32)
            nc.sync.dma_start(out=cc2_in[:], in_=PP[0:1, 0:1])
            nc.gpsimd.collective_compute(
                "AllReduce", ALU.add,
                replica_groups=[list(range(NCORES))],
                ins=[cc2_in.opt()], outs=[cc2_out.opt()])
            PT = sb.tile([1, 1], F32)
            nc.sync.dma_start(out=PT[0:1, 0:1], in_=cc2_out[:])
            MT = sb.tile([1, 1], F32)
            nc.vector.tensor_scalar(out=MT[:], in0=PT[0:1, 0:1],
                                    scalar1=-1.0 / FG, scalar2=1.0,
                                    op0=ALU.mult, op1=ALU.add)
            nc.sync.dma_start(out=out_e[:, :], in_=MT[:])
    nc.compile()
    return nc


def _get_compiled():
    global _compiled
    if _compiled is None:
        _compiled = _build()
    return _compiled


def kernel(logits, targets, _trace=False, _trace_kwargs=None):
    from concourse.bass_utils import run_bass_kernel_spmd

    logits = np.ascontiguousarray(np.asarray(logits), dtype=np.float32)
    targets = np.ascontiguousarray(np.asarray(targets), dtype=np.int32)
    fg = logits[:FG]
    bg = logits[FG:]
    tg = targets[FG:]
    fg_mat = fg.reshape(P, QB)   # query i at (p, blk) = p*QB + blk
    in_maps = []
    for r in range(NCORES):
        prq = np.concatenate([fg_mat[:, 2 * r], fg_mat[:, 2 * r + 1]])
        in_maps.append({
            "fgq": fg,
            "fgsh": np.ascontiguousarray(fg[r * FGS:(r + 1) * FGS]),
            "bgsh": np.ascontiguousarray(bg[r * BGS:(r + 1) * BGS]),
            "tgt": np.ascontiguousarray(tg[r * BGS:(r + 1) * BGS]),
            "prq": np.ascontiguousarray(prq),
        })
    nc = _get_compiled()
    kw = {}
    if _trace:
        kw = dict(trace=True, **(_trace_kwargs or {}))
    res = run_bass_kernel_spmd(nc, in_maps, core_ids=list(range(NCORES)), **kw)
    out = np.float32(res.results[0]["out"][0, 0])
    # metric = 1 - mean(prec) with prec in (0,1] is always in [0,1); an
    # out-of-range value means the device was left in a bad state by a
    # previously killed run -- retry once on a clean execution.
    if not (-1e-3 <= float(out) <= 1.0 + 1e-3):
        res = run_bass_kernel_spmd(nc, in_maps, core_ids=list(range(NCORES)), **kw)
        out = np.float32(res.results[0]["out"][0, 0])
    if _trace:
        return out, res
    return out


if __name__ == "__main__":
    rng = np.random.default_rng(0)
    logits = rng.standard_normal(N).astype(np.float32)
    targets = np.concatenate([np.ones(FG, np.int32), np.zeros(BG, np.int32)])
    print("metric:", kernel(logits, targets))


# revision 15
# speedup vs baseline: 3.3820x; 1.0332x over previous
"""Trainium2 Bass kernel for the AP-loss metric (nn_APLoss).

For N=262144 logits with the first FG=2048 being positives:
    metric = 1 - mean_i(prec_i),  prec_i = max{cur_j : v_j <= v_i}
    cur_i = a_i / (a_i + b_i)
    a_i = 0.5 + sum_fg clip((fg - v_i)/2 + .5, 0, 1)
    b_i = sum_{bg: target==0} clip((bg - v_i)/2 + .5, 0, 1)
(The reference's sorted scan + cummax is order-free; its bg>=min(fg)-1
threshold mask is provably redundant for the sums.)

Core identity: with u = x/2 and c_i = 0.5 - v_i/2,
    clip(c_i + u_j, 0, 1) = max(min(c_i, 1-u_j), -u_j) + u_j
so one fp16 4x-mode tensor_scalar per data column (queries broadcast along
the free axis, 1-u / -u as per-partition scalars) yields clip values minus
a u_j term whose sum is a per-query constant (sum_j u_j), added back to the
partials before the AllReduce. Invalid bg (target!=0) pinned to u=-500
still contributes exactly 0. TensorE sums the clip tiles along partitions
with accumulating matmul chains into PSUM (one 512-wide chain per PSUM
bank). bg and fg-data are 8-way sharded; one 16KB AllReduce combines
per-query partials; cur and the masked running max run per-core (each core
handles 2 of the 16 query blocks via the prq input), with a second scalar
AllReduce for the prec sum.
"""

import os
import sys

import numpy as np

sys.path.insert(0, "/opt/trn_rl_repo")

P = 128
FG = 2048
N = 262144
BG = N - FG
NCORES = 8
QB = FG // P            # 16 query blocks
FGS = FG // NCORES      # 256 fg data elems per core
BGS = BG // NCORES      # 32512 bg elems per core
NCOL = BGS // P         # 254 data columns per core
MMN = 512               # matmul moving-dim chunk (one PSUM bank)
COLSPLIT = 216          # b-columns in the early AllReduce (rest ride a late one)

_compiled = None


def _build():
    import concourse.bacc as bacc
    import concourse.tile as tile
    from concourse import mybir

    F32 = mybir.dt.float32
    F16 = mybir.dt.float16
    I32 = mybir.dt.int32
    ALU = mybir.AluOpType

    nc = bacc.Bacc("TRN2", target_bir_lowering=False, debug=False,
                   num_devices=NCORES)
    fgq_e = nc.declare_dram_parameter("fgq", [FG], F32, isOutput=False)
    fgsh_e = nc.declare_dram_parameter("fgsh", [FGS], F32, isOutput=False)
    bgsh_e = nc.declare_dram_parameter("bgsh", [BGS], F32, isOutput=False)
    tgt_e = nc.declare_dram_parameter("tgt", [BGS], I32, isOutput=False)
    prq_e = nc.declare_dram_parameter("prq", [2 * P], F32, isOutput=False)
    out_e = nc.declare_dram_parameter("out", [1, 1], F32, isOutput=True)

    with tile.TileContext(nc) as tc:
        with tc.tile_pool(name="sbuf", bufs=1) as sb, \
             tc.tile_pool(name="dram", bufs=1, space="DRAM") as dram, \
             tc.tile_pool(name="psum", bufs=1, space="PSUM") as ps:

            # ---------------- inputs ----------------
            F = sb.tile([P, QB], F32)         # all 2048 fg; query i at (p, blk)=p*QB+blk
            nc.sync.dma_start(out=F[:], in_=fgq_e[:].rearrange("(p c) -> p c", p=P))
            FS = sb.tile([P, FGS // P], F32)  # this core's fg data shard
            nc.sync.dma_start(out=FS[:], in_=fgsh_e[:].rearrange("(p c) -> p c", p=P))
            B = sb.tile([P, NCOL], F32)
            nc.sync.dma_start(out=B[:], in_=bgsh_e[:].rearrange("(p c) -> p c", p=P))
            T = sb.tile([P, NCOL], I32)
            nc.sync.dma_start(out=T[:], in_=tgt_e[:].rearrange("(p c) -> p c", p=P))

            # ---------------- query broadcast tile ----------------
            c0 = sb.tile([P, QB], F32)        # c0 = 0.5 - 0.5 v
            nc.vector.tensor_scalar(out=c0[:], in0=F[:], scalar1=-0.5,
                                    scalar2=0.5, op0=ALU.mult, op1=ALU.add)
            c0h = sb.tile([P, QB], F16)
            nc.vector.tensor_copy(c0h[:], c0[:])
            cq_d = dram.tile([FG], F16)       # (p,blk) order == query index order
            nc.sync.dma_start(out=cq_d[:].rearrange("(p c) -> p c", p=P),
                              in_=c0h[:])
            CQF = sb.tile([P, FG], F16)
            nc.sync.dma_start(out=CQF[:],
                              in_=cq_d[:].unsqueeze(0).broadcast_to([P, FG]))

            # ---------------- data preprocess ----------------
            # u_bg = target==0 ? 0.5*bg : -500 (-500 -> clip contributes 0)
            M = sb.tile([P, NCOL], F32)
            nc.vector.tensor_scalar(out=M[:], in0=T[:], scalar1=0,
                                    scalar2=None, op0=ALU.is_equal)
            T1 = sb.tile([P, NCOL], F32)
            nc.vector.tensor_scalar(out=T1[:], in0=B[:], scalar1=0.5,
                                    scalar2=500.0, op0=ALU.mult, op1=ALU.add)
            T2 = sb.tile([P, NCOL], F32)
            nc.vector.tensor_tensor(out=T2[:], in0=T1[:], in1=M[:], op=ALU.mult)
            U32 = sb.tile([P, NCOL], F32)
            nc.vector.tensor_scalar(out=U32[:], in0=T2[:], scalar1=-500.0,
                                    scalar2=None, op0=ALU.add)
            UF32 = sb.tile([P, FGS // P], F32)
            nc.vector.tensor_scalar(out=UF32[:], in0=FS[:], scalar1=0.5,
                                    scalar2=None, op0=ALU.mult)
            # per-partition scalars for the fused clip op
            UM1 = sb.tile([P, NCOL], F32)     # 1 - u
            nc.vector.tensor_scalar(out=UM1[:], in0=U32[:], scalar1=-1.0,
                                    scalar2=1.0, op0=ALU.mult, op1=ALU.add)
            UNEG = sb.tile([P, NCOL], F32)    # -u
            nc.vector.tensor_scalar(out=UNEG[:], in0=U32[:], scalar1=-1.0,
                                    scalar2=None, op0=ALU.mult)
            UFM1 = sb.tile([P, FGS // P], F32)
            nc.vector.tensor_scalar(out=UFM1[:], in0=UF32[:], scalar1=-1.0,
                                    scalar2=1.0, op0=ALU.mult, op1=ALU.add)
            UFNEG = sb.tile([P, FGS // P], F32)
            nc.vector.tensor_scalar(out=UFNEG[:], in0=UF32[:], scalar1=-1.0,
                                    scalar2=None, op0=ALU.mult)

            # sum_j u_j correction terms (per-query constants), as [128,1]
            # broadcasts via a DRAM bounce
            SUR = sb.tile([P, 1], F32)
            nc.vector.tensor_reduce(out=SUR[:], in_=U32[:],
                                    axis=mybir.AxisListType.X, op=ALU.add)
            SUFR = sb.tile([P, 1], F32)
            nc.vector.tensor_reduce(out=SUFR[:], in_=UF32[:],
                                    axis=mybir.AxisListType.X, op=ALU.add)
            SURT = sb.tile([1, P], F32)
            nc.sync.dma_start(out=SURT[0:1, :], in_=SUR[:, 0:1])
            SUFRT = sb.tile([1, P], F32)
            nc.sync.dma_start(out=SUFRT[0:1, :], in_=SUFR[:, 0:1])
            SUT = sb.tile([1, 1], F32)
            nc.vector.tensor_reduce(out=SUT[0:1, 0:1], in_=SURT[0:1, :],
                                    axis=mybir.AxisListType.X, op=ALU.add)
            SUFT = sb.tile([1, 1], F32)
            nc.vector.tensor_reduce(out=SUFT[0:1, 0:1], in_=SUFRT[0:1, :],
                                    axis=mybir.AxisListType.X, op=ALU.add)
            su_d = dram.tile([1], F32)
            nc.sync.dma_start(out=su_d[:], in_=SUT[0:1, 0:1])
            suf_d = dram.tile([1], F32)
            nc.sync.dma_start(out=suf_d[:], in_=SUFT[0:1, 0:1])
            SUB = sb.tile([P, 1], F32)
            nc.sync.dma_start(out=SUB[:],
                              in_=su_d[:].unsqueeze(0).broadcast_to([P, 1]))
            SUFB = sb.tile([P, 1], F32)
            nc.sync.dma_start(out=SUFB[:],
                              in_=suf_d[:].unsqueeze(0).broadcast_to([P, 1]))

            # ---------------- PE-summed clip tiles ----------------
            ONES16 = sb.tile([P, 1], F16)
            nc.vector.memset(ONES16[:], 1.0)
            PSB = ps.tile([1, FG], F32)       # 4 banks: bg cols < COLSPLIT
            with tc.tile_pool(name="psumA", bufs=1, space="PSUM") as psA:
                PSA = psA.tile([1, FG], F32)  # 4 banks: fg part (a - 0.5)
                for col in range(FGS // P):   # fg part first (finishes early)
                    YA = sb.tile([P, FG], F16, name="ya", tag="ya", bufs=2)
                    nc.vector.tensor_scalar(
                        out=YA[:], in0=CQF[:], scalar1=UFM1[:, col:col + 1],
                        scalar2=UFNEG[:, col:col + 1], op0=ALU.min, op1=ALU.max)
                    for m in range(FG // MMN):
                        nc.tensor.matmul(
                            PSA[0:1, m * MMN:(m + 1) * MMN],
                            lhsT=ONES16[:, 0:1],
                            rhs=YA[:, m * MMN:(m + 1) * MMN],
                            start=(col == 0), stop=(col == FGS // P - 1))
                AROW = sb.tile([1, FG], F32)
                nc.vector.tensor_copy(AROW[:], PSA[0:1, :])
            with tc.tile_pool(name="psumB2", bufs=1, space="PSUM") as psB2:
                PSB2 = psB2.tile([1, FG], F32)  # 4 banks: bg cols >= COLSPLIT
                for col in range(NCOL):
                    Y2 = sb.tile([P, FG], F16, name="y2", tag="y2", bufs=4)
                    nc.vector.tensor_scalar(
                        out=Y2[:], in0=CQF[:], scalar1=UM1[:, col:col + 1],
                        scalar2=UNEG[:, col:col + 1], op0=ALU.min, op1=ALU.max)
                    tgt_ps = PSB if col < COLSPLIT else PSB2
                    for m in range(FG // MMN):
                        nc.tensor.matmul(
                            tgt_ps[0:1, m * MMN:(m + 1) * MMN],
                            lhsT=ONES16[:, 0:1],
                            rhs=Y2[:, m * MMN:(m + 1) * MMN],
                            start=(col in (0, COLSPLIT)),
                            stop=(col in (COLSPLIT - 1, NCOL - 1)))
                BROW = sb.tile([1, FG], F32)
                nc.vector.tensor_copy(BROW[:], PSB[0:1, :])
                BROW2 = sb.tile([1, FG], F32)
                nc.vector.tensor_copy(BROW2[:], PSB2[0:1, :])

            # ------- merge + corrections + overlapped/late AllReduces -------
            PEB = sb.tile([P, QB], F32)
            nc.sync.dma_start(out=PEB[:], in_=BROW[0:1, :])
            AFG = sb.tile([P, QB], F32)
            nc.sync.dma_start(out=AFG[:], in_=AROW[0:1, :])
            PEBc = sb.tile([P, QB], F32)
            nc.vector.tensor_scalar(out=PEBc[:], in0=PEB[:],
                                    scalar1=SUB[:, 0:1], scalar2=None,
                                    op0=ALU.add)
            AFGc = sb.tile([P, QB], F32)
            nc.vector.tensor_scalar(out=AFGc[:], in0=AFG[:],
                                    scalar1=SUFB[:, 0:1], scalar2=None,
                                    op0=ALU.add)
            PEB2 = sb.tile([P, QB], F32)
            nc.sync.dma_start(out=PEB2[:], in_=BROW2[0:1, :])

            cc_in = dram.tile([2 * FG], F32)
            cc_out = dram.tile([2 * FG], F32)
            nc.sync.dma_start(out=cc_in[0:FG].rearrange("(p c) -> p c", p=P),
                              in_=PEBc[:])
            nc.sync.dma_start(out=cc_in[FG:2 * FG].rearrange("(p c) -> p c", p=P),
                              in_=AFGc[:])
            nc.gpsimd.collective_compute(
                "AllReduce", ALU.add,
                replica_groups=[list(range(NCORES))],
                ins=[cc_in.opt()], outs=[cc_out.opt()])
            cc_inB = dram.tile([FG], F32)
            cc_outB = dram.tile([FG], F32)
            nc.sync.dma_start(out=cc_inB[:].rearrange("(p c) -> p c", p=P),
                              in_=PEB2[:])
            nc.gpsimd.collective_compute(
                "AllReduce", ALU.add,
                replica_groups=[list(range(NCORES))],
                ins=[cc_inB.opt()], outs=[cc_outB.opt()])
            BTA = sb.tile([P, QB], F32)
            nc.sync.dma_start(out=BTA[:],
                              in_=cc_out[0:FG].rearrange("(p c) -> p c", p=P))
            AT = sb.tile([P, QB], F32)
            nc.sync.dma_start(out=AT[:],
                              in_=cc_out[FG:2 * FG].rearrange("(p c) -> p c", p=P))
            BTB = sb.tile([P, QB], F32)
            nc.sync.dma_start(out=BTB[:],
                              in_=cc_outB[:].rearrange("(p c) -> p c", p=P))
            BT = sb.tile([P, QB], F32)
            nc.vector.tensor_add(BT[:], BTA[:], BTB[:])

            # ---------------- cur = a / (a + b) ----------------
            Aq = sb.tile([P, QB], F32)
            nc.vector.tensor_scalar(out=Aq[:], in0=AT[:], scalar1=0.5,
                                    scalar2=None, op0=ALU.add)
            Sq = sb.tile([P, QB], F32)
            nc.vector.tensor_add(Sq[:], Aq[:], BT[:])
            RS = sb.tile([P, QB], F32)
            nc.vector.reciprocal(RS[:], Sq[:])
            CUR = sb.tile([P, QB], F32)
            nc.vector.tensor_tensor(out=CUR[:], in0=Aq[:], in1=RS[:], op=ALU.mult)

            # ------- prec = masked running max (2 query blocks per core) -------
            VQH = sb.tile([P, QB], F16)       # fp16-rounded queries: self-compare safe
            nc.vector.tensor_copy(VQH[:], F[:])
            CUR16 = sb.tile([P, QB], F16)
            nc.vector.tensor_copy(CUR16[:], CUR[:])

            vline = dram.tile([FG], F16)
            nc.sync.dma_start(out=vline[:].rearrange("(p c) -> p c", p=P),
                              in_=VQH[:])
            cline = dram.tile([FG], F16)
            nc.sync.dma_start(out=cline[:].rearrange("(p c) -> p c", p=P),
                              in_=CUR16[:])
            VB = sb.tile([P, FG], F16)
            nc.sync.dma_start(out=VB[:],
                              in_=vline[:].unsqueeze(0).broadcast_to([P, FG]))
            CB = sb.tile([P, FG], F16)
            nc.sync.dma_start(out=CB[:],
                              in_=cline[:].unsqueeze(0).broadcast_to([P, FG]))

            # this core's 2 query blocks (input-sharded), fp16-rounded like VB
            PRQ = sb.tile([P, 2], F32)
            nc.sync.dma_start(out=PRQ[:], in_=prq_e[:].rearrange("(c p) -> p c", p=P))
            PRQH = sb.tile([P, 2], F16)
            nc.vector.tensor_copy(PRQH[:], PRQ[:])
            PRQR = sb.tile([P, 2], F32)
            nc.vector.tensor_copy(PRQR[:], PRQH[:])

            PREC = sb.tile([P, 2], F32)
            MSK = sb.tile([P, FG], F16)
            TM = sb.tile([P, FG], F16)
            for blk in range(2):
                nc.vector.tensor_scalar(out=MSK[:], in0=VB[:],
                                        scalar1=PRQR[:, blk:blk + 1],
                                        scalar2=-10000.0,
                                        op0=ALU.is_gt, op1=ALU.mult)
                nc.vector.tensor_tensor(out=TM[:], in0=MSK[:], in1=CB[:],
                                        op=ALU.add)
                nc.vector.tensor_reduce(out=PREC[:, blk:blk + 1], in_=TM[:],
                                        axis=mybir.AxisListType.X, op=ALU.max)

            # --- metric = 1 - sum(prec)/FG  (partition sum via DMA transpose) ---
            PSUM_ = sb.tile([P, 1], F32)
            nc.vector.tensor_reduce(out=PSUM_[:], in_=PREC[:],
                                    axis=mybir.AxisListType.X, op=ALU.add)
            PSUMT = sb.tile([1, P], F32)
            nc.sync.dma_start(out=PSUMT[0:1, :], in_=PSUM_[:, 0:1])
            PP = sb.tile([1, 1], F32)
            nc.vector.tensor_reduce(out=PP[0:1, 0:1], in_=PSUMT[0:1, :],
                                    axis=mybir.AxisListType.X, op=ALU.add)
            cc2_in = dram.tile([1], F32)
            cc2_out = dram.tile([1], F32)
            nc.sync.dma_start(out=cc2_in[:], in_=PP[0:1, 0:1])
            nc.gpsimd.collective_compute(
                "AllReduce", ALU.add,
                replica_groups=[list(range(NCORES))],
                ins=[cc2_in.opt()], outs=[cc2_out.opt()])
            PT = sb.tile([1, 1], F32)
            nc.sync.dma_start(out=PT[0:1, 0:1], in_=cc2_out[:])
            MT = sb.tile([1, 1], F32)
            nc.vector.tensor_scalar(out=MT[:], in0=PT[0:1, 0:1],
                                    scalar1=-1.0 / FG, scalar2=1.0,
                                    op0=ALU.mult, op1=ALU.add)
            nc.sync.dma_start(out=out_e[:, :], in_=MT[:])
    nc.compile()
    return nc


def _get_compiled():
    global _compiled
    if _compiled is None:
        _compiled = _build()
    return _compiled


def kernel(logits, targets, _trace=False, _trace_kwargs=None):
    from concourse.bass_utils import run_bass_kernel_spmd

    logits = np.ascontiguousarray(np.asarray(logits), dtype=np.float32)
    targets = np.ascontiguousarray(np.asarray(targets), dtype=np.int32)
    fg = logits[:FG]
    bg = logits[FG:]
    tg = targets[FG:]
    fg_mat = fg.reshape(P, QB)   # query i at (p, blk) = p*QB + blk
    in_maps = []
    for r in range(NCORES):
        prq = np.concatenate([fg_mat[:, 2 * r], fg_mat[:, 2 * r + 1]])
        in_maps.append({
            "fgq": fg,
            "fgsh": np.ascontiguousarray(fg[r * FGS:(r + 1) * FGS]),
            "bgsh": np.ascontiguousarray(bg[r * BGS:(r + 1) * BGS]),
            "tgt": np.ascontiguousarray(tg[r * BGS:(r + 1) * BGS]),
            "prq": np.ascontiguousarray(prq),
        })
    nc = _get_compiled()
    kw = {}
    if _trace:
        kw = dict(trace=True, **(_trace_kwargs or {}))
    res = run_bass_kernel_spmd(nc, in_maps, core_ids=list(range(NCORES)), **kw)
    out = np.float32(res.results[0]["out"][0, 0])
    # metric = 1 - mean(prec) with prec in (0,1] is always in [0,1); an
    # out-of-range value means the device was left in a bad state by a
    # previously killed run -- retry once on a clean execution.
    if not (-1e-3 <= float(out) <= 1.0 + 1e-3):
        res = run_bass_kernel_spmd(nc, in_maps, core_ids=list(range(NCORES)), **kw)
        out = np.float32(res.results[0]["out"][0, 0])
    if _trace:
        return out, res
    return out


if __name__ == "__main__":
    rng = np.random.default_rng(0)
    logits = rng.standard_normal(N).astype(np.float32)
    targets = np.concatenate([np.ones(FG, np.int32), np.zeros(BG, np.int32)])
    print("metric:", kernel(logits, targets))
